# revision 53
# baseline (speedup 1.0000x reference)
"""3-level db4 DWT (circular, stride-2) over x:(32, 8192, 64) on 8 TRN2 NeuronCores.

Data-parallel over batch (4 batches/core). Per core the (seq, b*c=256) matrix
is processed as banded matmuls on the TensorEngine. v3 scheme: per U-step two
mixed A/D psum tiles are built with 2 accumulating matmuls each (N=512):

  psE = S_M^T  @ mv_e + S_B^T  @ mv_o     (A of even 64-blocks in parts 0:64,
                                           D of even blocks in parts 64:128)
  psO = S_Msw^T@ mv_o + S_Bsw^T@ mv_b     (swapped: D odd in 0:64, A odd in 64:128)

where mv_e/mv_o/mv_b are strided 2-block (128,2,256) views of the tile-major
input storage (block j holds seq [128j,128j+128)), and S_B/S_Bsw carry the
6-tap block-boundary spill (circular wrap handled by a padded copy of block 0).
Each input element streams through the PE exactly twice (A+D combined) —
the minimum for this banded scheme with 128-part psum alignment.

A-coeffs land partition-aligned: psE[0:64]/psO[64:128] copy straight into the
next level's tile-major storage (DVE owns psE, ACT owns psO — no psum bank is
touched by two engines). D-coeffs (and final-level psums) stage through SBUF
as fp16 in a scrambled-but-regular layout that the host unscrambles.

Self-contained: hardcodes shapes/filters; needs numpy + concourse (axon TRN2).
"""

import numpy as np

import concourse.bass as bass  # noqa: F401
import concourse.mybir as mybir
from concourse import bacc
from concourse.tile import TileContext
from concourse import bass2jax

# db4 decomposition filters (pywt convention) — matches the reference.
DEC_LO = np.array([-0.010597401784997278, 0.032883011666982945, 0.030841381835986965,
                   -0.18703481171888114, -0.02798376941698385, 0.6308807679295904,
                   0.7148465705525415, 0.23037781330885523], dtype=np.float32)
DEC_HI = np.array([-0.23037781330885523, 0.7148465705525415, -0.6308807679295904,
                   -0.02798376941698385, 0.18703481171888114, 0.030841381835986965,
                   0.032883011666982945, -0.010597401784997278], dtype=np.float32)

N_CORES = 8
B = 32             # total batch
BC = B // N_CORES  # batches per core
C = 64             # channels
CB = BC * C        # 256 moving columns per core
N0 = 8192          # level-1 sequence length
W = 2 * CB         # 512: psum free width / matmul N

# mm dtype: "float16" (1 cyc/row on PE, 11-bit mantissa — best speed/accuracy
# for N(0,1) data), "float32r" (fp32 bits, ~2x slower), or "bfloat16".
MM_DTYPE = "float16"

N_UNITS = [16, 8, 4]   # U-steps per level (= n_level / 512)
USEC = 4               # level-1 U-steps per input DMA section


def _np_dt(name):
    if name == "bfloat16":
        import ml_dtypes
        return ml_dtypes.bfloat16
    if name == "float16":
        return np.float16
    return np.float32


def make_stationaries():
    """(128, 384) concat of S_M, S_B, S_Mh.

    S_M[s, m<64] = lo-taps (A), S_M[s, 64+m'] = hi-taps (D), band s = 2m+j.
    S_B = the 6-tap spill into the next block (rows 0..5 used).
    S_Mh = S_M's rows 64:128 rebased to partitions 0:64 (for K=64 levels —
    hardware rejects accumulation groups that mix partition bases)."""
    S_M = np.zeros((128, 128), np.float32)
    S_B = np.zeros((128, 128), np.float32)
    for m in range(64):
        for j in range(8):
            s = 2 * m + j
            if s < 128:
                S_M[s, m] = DEC_LO[j]
                S_M[s, m + 64] = DEC_HI[j]
            else:              # spill into next block's rows 0..5
                S_B[s - 128, m] = DEC_LO[j]
                S_B[s - 128, m + 64] = DEC_HI[j]
    S_Mh = np.zeros((128, 128), np.float32)
    S_Mh[0:64, :] = S_M[64:128, :]
    return np.concatenate([S_M, S_B, S_Mh], axis=1)


def build_bass(repeat=1, hw_loop=0):
    DT = getattr(mybir.dt, MM_DTYPE)
    f32 = mybir.dt.float32
    nc = bacc.Bacc(trn_type="TRN2", target_bir_lowering=False, num_devices=N_CORES)

    # input: tile-major (block j at cols CB*j) + 2 wrap-pad blocks -> 66 blocks
    xa_d = nc.dram_tensor("xa", [128, 66 * CB], DT, kind="ExternalInput")
    sm_d = nc.dram_tensor("smats", [128, 3 * 128], DT, kind="ExternalInput")
    d1_d = nc.dram_tensor("d1", [128, 32 * CB], DT, kind="ExternalOutput")
    d2_d = nc.dram_tensor("d2", [128, 16 * CB], DT, kind="ExternalOutput")
    e3_d = nc.dram_tensor("e3", [128, 8 * CB], DT, kind="ExternalOutput")
    o3_d = nc.dram_tensor("o3", [128, 8 * CB], DT, kind="ExternalOutput")

    with TileContext(nc) as tc:
        with (
            tc.tile_pool(name="const", bufs=1) as cpool,
            tc.tile_pool(name="data", bufs=1) as dpool,
            tc.tile_pool(name="stage", bufs=6) as spool,
            tc.tile_pool(name="psum", bufs=4, space="PSUM") as ppool,
        ):
            S = cpool.tile([128, 3 * 128], DT)
            nc.sync.dma_start(out=S, in_=sm_d[:])
            S_M, S_B, S_Mh = S[:, 0:128], S[:, 128:256], S[:, 256:384]

            # level-1 input sections (first ones smaller so matmuls start
            # sooner); each carries 2 overlap blocks for the boundary matmul
            SEC_US = [1, 1, 2, 4, 4, 4]         # U-steps per section
            SEC_START = [0, 1, 2, 4, 8, 12]
            xsecs = []
            for s, su in enumerate(SEC_US):
                xsecs.append(dpool.tile([128, (4 * su + 2) * CB], DT,
                                        tag=f"xs{s}", name=f"xs{s}"))

            def sec_of(U):
                for s in range(len(SEC_US) - 1, -1, -1):
                    if U >= SEC_START[s]:
                        return s
                raise AssertionError
            # per-level psum evacuation stores (tile-major, +1 wrap pad block,
            # +1 slack block so the (g t) view splits evenly).
            # E[0:64, CB*j:+CB] = lower half of next-level tile j (A-coeffs),
            # O[64:128, ...]    = upper half; E[64:128]/O[0:64] = D-coeffs.
            E1 = dpool.tile([128, 34 * CB], DT)
            O1 = dpool.tile([128, 34 * CB], DT)
            E2 = dpool.tile([128, 18 * CB], DT)
            O2 = dpool.tile([128, 18 * CB], DT)
            E3 = dpool.tile([128, 8 * CB], DT)
            O3 = dpool.tile([128, 8 * CB], DT)
            EO = [(E1, O1), (E2, O2), (E3, O3)]

            def dma_in():
                for s, su in enumerate(SEC_US):
                    c = 4 * SEC_START[s] * CB
                    nc.sync.dma_start(out=xsecs[s],
                                      in_=xa_d[:, c:c + (4 * su + 2) * CB])

            def gview(t):
                return t.rearrange("p (g t c) -> p g t c", t=2, c=CB)

            def do_unit(lvl, U, d_dram):
                n_units = N_UNITS[lvl]
                Edst, Odst = EO[lvl]
                if True:
                    if lvl == 0:
                        s = sec_of(U)
                        g0 = 2 * (U - SEC_START[s])
                    else:
                        g0 = 2 * U
                    psE = ppool.tile([128, W], f32, tag="psE")
                    psO = ppool.tile([128, W], f32, tag="psO")
                    if lvl == 0:
                        v = gview(xsecs[s])
                        mv_e = v[:, g0:g0 + 2, 0, :]
                        mv_o = v[:, g0:g0 + 2, 1, :]
                        mv_b = v[:, g0 + 1:g0 + 3, 0, :]
                        nc.tensor.matmul(psE, S_M, mv_e, start=True, stop=False)
                        nc.tensor.matmul(psE, S_B, mv_o, start=False, stop=True)
                        nc.tensor.matmul(psO, S_M, mv_o, start=True, stop=False)
                        nc.tensor.matmul(psO, S_B, mv_b, start=False, stop=True)
                    else:
                        # inputs live split across Eprev (lower halves of the
                        # next-level tiles, partitions 0:64) and Oprev (upper
                        # halves, also at partitions 0:64) -> K=64 matmuls,
                        # all base-0 (mixed-base accumulation is rejected by HW)
                        Ep, Op = EO[lvl - 1]
                        ve = gview(Ep)[0:64, :, :, :]
                        vo = gview(Op)[0:64, :, :, :]
                        e_ev = ve[:, g0:g0 + 2, 0, :]
                        e_od = ve[:, g0:g0 + 2, 1, :]
                        e_sh = ve[:, g0 + 1:g0 + 3, 0, :]
                        o_ev = vo[:, g0:g0 + 2, 0, :]
                        o_od = vo[:, g0:g0 + 2, 1, :]
                        nc.tensor.matmul(psE, S_M[0:64, :], e_ev,
                                         start=True, stop=False)
                        nc.tensor.matmul(psE, S_Mh[0:64, :], o_ev,
                                         start=False, stop=False)
                        nc.tensor.matmul(psE, S_B[0:64, :], e_od,
                                         start=False, stop=True)
                        nc.tensor.matmul(psO, S_M[0:64, :], e_od,
                                         start=True, stop=False)
                        nc.tensor.matmul(psO, S_Mh[0:64, :], o_od,
                                         start=False, stop=False)
                        nc.tensor.matmul(psO, S_B[0:64, :], e_sh,
                                         start=False, stop=True)
                    c0 = W * U
                    # one full-width evacuation per psum tile (psum-read cost
                    # is per free-col: never split a psum copy by partitions)
                    nc.vector.tensor_copy(out=Edst[:, c0:c0 + W], in_=psE[:])
                    nc.scalar.copy(out=Odst[:, c0:c0 + W], in_=psO[:])
                    if lvl < 2 and U == 0:   # circular wrap pad block
                        cp = CB * (2 * n_units)
                        nc.vector.tensor_copy(out=Edst[:, cp:cp + CB],
                                              in_=psE[:, 0:CB])
                        nc.scalar.copy(out=Odst[:, cp:cp + CB],
                                       in_=psO[:, 0:CB])
                    # outputs stream out per U-pair (levels 1-2, in-order) or
                    # per unit (level 3, whose units are emitted out of order)
                    if d_dram is not None:
                        if U % 2 == 1:
                            c1 = W * (U - 1)
                            nc.sync.dma_start(out=d_dram[64:128, c1:c1 + 2 * W],
                                              in_=Edst[64:128, c1:c1 + 2 * W])
                            nc.sync.dma_start(out=d_dram[0:64, c1:c1 + 2 * W],
                                              in_=Odst[64:128, c1:c1 + 2 * W])
                    else:
                        nc.sync.dma_start(out=e3_d[:, c0:c0 + W],
                                          in_=Edst[:, c0:c0 + W])
                        nc.sync.dma_start(out=o3_d[:, c0:c0 + W],
                                          in_=Odst[:, c0:c0 + W])

            wtile = cpool.tile([128, 128], DT)

            def whole():
                # PE warmup first (depends only on a memset): dummy matmuls
                # ramp the clock gate to full rate while the input sections
                # are still streaming in.
                nc.vector.memset(wtile, 0.0)
                wps = ppool.tile([128, W], f32, tag="psE", name="wps")
                for _ in range(14):
                    nc.tensor.matmul(wps[:, 0:128], wtile, wtile,
                                     start=True, stop=True)
                dma_in()
                # interleave levels: a unit is emitted as soon as its inputs
                # exist, so levels 2/3 fill level-1's DMA-stall gaps and the
                # kernel tail stays short. L3-0 goes last (its inputs are
                # ready long before) to keep the final dependency chain flat.
                for U in range(N_UNITS[0]):
                    do_unit(0, U, d1_d)
                    if U >= 2 and U % 2 == 0:
                        U2 = (U - 2) // 2
                        do_unit(1, U2, d2_d)
                        if U2 in (4, 6):
                            do_unit(2, U2 // 2 - 1, None)
                do_unit(1, 7, d2_d)
                do_unit(2, 3, None)
                do_unit(2, 0, None)

            if hw_loop:
                with tc.For_i(0, hw_loop, 1):
                    whole()
            else:
                for _ in range(repeat):
                    whole()

    nc.compile()
    return nc


_BUILD_CACHE = {}


def _get_runner(repeat=1, hw_loop=0):
    """Build (once) and return a jitted SPMD runner: fn(in_maps) -> results."""
    key = (MM_DTYPE, repeat, hw_loop)
    if key in _BUILD_CACHE:
        return _BUILD_CACHE[key]

    import jax
    from jax.sharding import Mesh, PartitionSpec
    from jax.experimental.shard_map import shard_map

    nc = build_bass(repeat, hw_loop)
    bass2jax.install_neuronx_cc_hook()

    partition_name = nc.partition_id_tensor.name if nc.partition_id_tensor else None
    in_names, out_names, out_avals, zero_outs = [], [], [], []
    for alloc in nc.m.functions[0].allocations:
        if not isinstance(alloc, mybir.MemoryLocationSet):
            continue
        name = alloc.memorylocations[0].name
        if alloc.kind == "ExternalInput":
            if name != partition_name:
                in_names.append(name)
        elif alloc.kind == "ExternalOutput":
            out_names.append(name)
            shape = tuple(alloc.tensor_shape)
            dtype = mybir.dt.np(alloc.dtype)
            out_avals.append(jax.core.ShapedArray(shape, dtype))
            zero_outs.append(np.zeros(shape, dtype))
    n_params = len(in_names)
    n_outs = len(out_avals)
    all_in_names = list(in_names) + list(out_names)
    if partition_name is not None:
        all_in_names.append(partition_name)
    donate = tuple(range(n_params, n_params + n_outs))

    def _body(*args):
        operands = list(args)
        if partition_name is not None:
            operands.append(bass2jax.partition_id_tensor())
        outs = bass2jax._bass_exec_p.bind(
            *operands,
            out_avals=tuple(out_avals),
            in_names=tuple(all_in_names),
            out_names=tuple(out_names),
            lowering_input_output_aliases=(),
            sim_require_finite=True,
            sim_require_nnan=True,
            nc=nc,
        )
        return tuple(outs)

    devices = jax.devices()[:N_CORES]
    mesh = Mesh(np.asarray(devices), ("core",))
    in_specs = (PartitionSpec("core"),) * (n_params + n_outs)
    out_specs = (PartitionSpec("core"),) * len(out_names)
    sharded = jax.jit(
        shard_map(_body, mesh=mesh, in_specs=in_specs, out_specs=out_specs,
                  check_rep=False),
        donate_argnums=donate, keep_unused=True,
    )

    def run(in_maps, raw=False):
        per_core = [[np.asarray(m[name]) for name in in_names] for m in in_maps]
        concat_in = [np.concatenate([per_core[c][i] for c in range(N_CORES)], axis=0)
                     for i in range(n_params)]
        concat_zeros = [np.zeros((N_CORES * z.shape[0], *z.shape[1:]), z.dtype)
                        for z in zero_outs]
        out_arrs = sharded(*concat_in, *concat_zeros)
        if raw:
            return out_arrs
        return [
            {name: np.asarray(out_arrs[i]).reshape(N_CORES, *out_avals[i].shape)[c]
             for i, name in enumerate(out_names)}
            for c in range(N_CORES)
        ]

    run.sharded = sharded
    run.in_names = in_names
    run.out_names = out_names
    run.out_avals = out_avals
    run.zero_outs = zero_outs
    run.nc = nc
    _BUILD_CACHE[key] = run
    return run


def _prep_core(x2d, np_dt):
    """x2d (8192, CB) fp32 -> xa (128, 66*CB) tile-major + 2 wrap-pad blocks."""
    t = x2d.reshape(64, 128, CB)
    t = np.concatenate([t, t[0:2]], axis=0)      # 66 blocks
    return np.ascontiguousarray(t.transpose(1, 0, 2).reshape(128, 66 * CB)).astype(np_dt)


def _unscramble_d(arr):
    """(128, n_units*512) fp16 scrambled D -> (n_pos, BC, C) fp32 seq-major."""
    n_units = arr.shape[1] // W
    a = arr.astype(np.float32).reshape(128, n_units, 2, CB)
    lo, hi = a[:64], a[64:]          # lo: odd blocks, hi: even blocks
    # out block k=4U+2h+t: t=0 -> hi, t=1 -> lo ; shape (U, h, t, 64, CB)
    blocks = np.stack([hi, lo], axis=3)            # (64, U, 2, 2, CB)
    blocks = blocks.transpose(1, 2, 3, 0, 4)       # (U, h, t, 64, CB)
    seq = blocks.reshape(n_units * 256, BC, C)
    return seq


def _unscramble_ad(e3, o3):
    """Final level mixed psum stages -> (A3_seq, D3_seq) each (n_pos, BC, C)."""
    n_units = e3.shape[1] // W
    e = e3.astype(np.float32).reshape(128, n_units, 2, CB)
    o = o3.astype(np.float32).reshape(128, n_units, 2, CB)
    A_even, D_even = e[:64], e[64:]
    A_odd, D_odd = o[:64], o[64:]
    A = np.stack([A_even, A_odd], axis=3).transpose(1, 2, 3, 0, 4)
    D = np.stack([D_even, D_odd], axis=3).transpose(1, 2, 3, 0, 4)
    return (A.reshape(n_units * 256, BC, C), D.reshape(n_units * 256, BC, C))


def kernel(x):
    x = np.asarray(x, dtype=np.float32)
    assert x.shape == (B, N0, C)
    np_dt = _np_dt(MM_DTYPE)
    smats = make_stationaries().astype(np_dt)

    in_maps = []
    for i in range(N_CORES):
        xc = x[BC * i:BC * (i + 1)]                  # (BC, 8192, C)
        x2d = xc.transpose(1, 0, 2).reshape(N0, CB)  # (seq, cb)
        in_maps.append({"xa": _prep_core(x2d, np_dt), "smats": smats})

    res = _get_runner()(in_maps)

    A3 = np.empty((B, N0 // 8, C), np.float32)
    D3 = np.empty((B, N0 // 8, C), np.float32)
    D2 = np.empty((B, N0 // 4, C), np.float32)
    D1 = np.empty((B, N0 // 2, C), np.float32)
    for i in range(N_CORES):
        sl = slice(BC * i, BC * (i + 1))
        a3s, d3s = _unscramble_ad(np.asarray(res[i]["e3"]), np.asarray(res[i]["o3"]))
        A3[sl] = a3s.transpose(1, 0, 2)
        D3[sl] = d3s.transpose(1, 0, 2)
        D2[sl] = _unscramble_d(np.asarray(res[i]["d2"])).transpose(1, 0, 2)
        D1[sl] = _unscramble_d(np.asarray(res[i]["d1"])).transpose(1, 0, 2)
    return (A3, D3, D2, D1)


# revision 56
# speedup vs baseline: 1.4101x; 1.4101x over previous
"""3-level db4 DWT (circular, stride-2) over x:(32, 8192, 64) on 8 TRN2 NeuronCores.

Data-parallel over batch (4 batches/core). Per core the (seq, b*c=256) matrix
is processed as banded matmuls on the TensorEngine. v3 scheme: per U-step two
mixed A/D psum tiles are built with 2 accumulating matmuls each (N=512):

  psE = S_M^T  @ mv_e + S_B^T  @ mv_o     (A of even 64-blocks in parts 0:64,
                                           D of even blocks in parts 64:128)
  psO = S_Msw^T@ mv_o + S_Bsw^T@ mv_b     (swapped: D odd in 0:64, A odd in 64:128)

where mv_e/mv_o/mv_b are strided 2-block (128,2,256) views of the tile-major
input storage (block j holds seq [128j,128j+128)), and S_B/S_Bsw carry the
6-tap block-boundary spill (circular wrap handled by a padded copy of block 0).
Each input element streams through the PE exactly twice (A+D combined) —
the minimum for this banded scheme with 128-part psum alignment.

A-coeffs land partition-aligned: psE[0:64]/psO[64:128] copy straight into the
next level's tile-major storage (DVE owns psE, ACT owns psO — no psum bank is
touched by two engines). D-coeffs (and final-level psums) stage through SBUF
as fp16 in a scrambled-but-regular layout that the host unscrambles.

Self-contained: hardcodes shapes/filters; needs numpy + concourse (axon TRN2).
"""

import numpy as np

import concourse.bass as bass  # noqa: F401
import concourse.mybir as mybir
from concourse import bacc
from concourse.tile import TileContext
from concourse import bass2jax

# db4 decomposition filters (pywt convention) — matches the reference.
DEC_LO = np.array([-0.010597401784997278, 0.032883011666982945, 0.030841381835986965,
                   -0.18703481171888114, -0.02798376941698385, 0.6308807679295904,
                   0.7148465705525415, 0.23037781330885523], dtype=np.float32)
DEC_HI = np.array([-0.23037781330885523, 0.7148465705525415, -0.6308807679295904,
                   -0.02798376941698385, 0.18703481171888114, 0.030841381835986965,
                   0.032883011666982945, -0.010597401784997278], dtype=np.float32)

N_CORES = 8
B = 32             # total batch
BC = B // N_CORES  # batches per core
C = 64             # channels
CB = BC * C        # 256 moving columns per core
N0 = 8192          # level-1 sequence length
W = 2 * CB         # 512: psum free width / matmul N

# mm dtype: "float16" (1 cyc/row on PE, 11-bit mantissa — best speed/accuracy
# for N(0,1) data), "float32r" (fp32 bits, ~2x slower), or "bfloat16".
MM_DTYPE = "float16"

N_UNITS = [16, 8, 4]   # U-steps per level (= n_level / 512)
USEC = 4               # level-1 U-steps per input DMA section


def _np_dt(name):
    if name == "bfloat16":
        import ml_dtypes
        return ml_dtypes.bfloat16
    if name == "float16":
        return np.float16
    return np.float32


def make_stationaries():
    """(128, 384) concat of S_M, S_B, S_Mh.

    S_M[s, m<64] = lo-taps (A), S_M[s, 64+m'] = hi-taps (D), band s = 2m+j.
    S_B = the 6-tap spill into the next block (rows 0..5 used).
    S_Mh = S_M's rows 64:128 rebased to partitions 0:64 (for K=64 levels —
    hardware rejects accumulation groups that mix partition bases)."""
    S_M = np.zeros((128, 128), np.float32)
    S_B = np.zeros((128, 128), np.float32)
    for m in range(64):
        for j in range(8):
            s = 2 * m + j
            if s < 128:
                S_M[s, m] = DEC_LO[j]
                S_M[s, m + 64] = DEC_HI[j]
            else:              # spill into next block's rows 0..5
                S_B[s - 128, m] = DEC_LO[j]
                S_B[s - 128, m + 64] = DEC_HI[j]
    S_Mh = np.zeros((128, 128), np.float32)
    S_Mh[0:64, :] = S_M[64:128, :]
    return np.concatenate([S_M, S_B, S_Mh], axis=1)


def build_bass(repeat=1, hw_loop=0):
    DT = getattr(mybir.dt, MM_DTYPE)
    f32 = mybir.dt.float32
    nc = bacc.Bacc(trn_type="TRN2", target_bir_lowering=False, num_devices=N_CORES)

    # input: tile-major (block j at cols CB*j) + 2 wrap-pad blocks -> 66 blocks
    xa_d = nc.dram_tensor("xa", [128, 66 * CB], DT, kind="ExternalInput")
    sm_d = nc.dram_tensor("smats", [128, 3 * 128], DT, kind="ExternalInput")
    d1_d = nc.dram_tensor("d1", [128, 32 * CB], DT, kind="ExternalOutput")
    d2_d = nc.dram_tensor("d2", [128, 16 * CB], DT, kind="ExternalOutput")
    e3_d = nc.dram_tensor("e3", [128, 8 * CB], DT, kind="ExternalOutput")
    o3_d = nc.dram_tensor("o3", [128, 8 * CB], DT, kind="ExternalOutput")

    with TileContext(nc) as tc:
        with (
            tc.tile_pool(name="const", bufs=1) as cpool,
            tc.tile_pool(name="data", bufs=1) as dpool,
            tc.tile_pool(name="stage", bufs=6) as spool,
            tc.tile_pool(name="psum", bufs=4, space="PSUM") as ppool,
        ):
            S = cpool.tile([128, 3 * 128], DT)
            nc.sync.dma_start(out=S, in_=sm_d[:])
            S_M, S_B, S_Mh = S[:, 0:128], S[:, 128:256], S[:, 256:384]

            # level-1 input sections (first ones smaller so matmuls start
            # sooner); each carries 2 overlap blocks for the boundary matmul
            SEC_US = [1, 1, 2, 4, 4, 4]         # U-steps per section
            SEC_START = [0, 1, 2, 4, 8, 12]
            xsecs = []
            for s, su in enumerate(SEC_US):
                xsecs.append(dpool.tile([128, (4 * su + 2) * CB], DT,
                                        tag=f"xs{s}", name=f"xs{s}"))

            def sec_of(U):
                for s in range(len(SEC_US) - 1, -1, -1):
                    if U >= SEC_START[s]:
                        return s
                raise AssertionError
            # per-level psum evacuation stores (tile-major, +1 wrap pad block,
            # +1 slack block so the (g t) view splits evenly).
            # E[0:64, CB*j:+CB] = lower half of next-level tile j (A-coeffs),
            # O[64:128, ...]    = upper half; E[64:128]/O[0:64] = D-coeffs.
            E1 = dpool.tile([128, 34 * CB], DT)
            O1 = dpool.tile([128, 34 * CB], DT)
            E2 = dpool.tile([128, 18 * CB], DT)
            O2 = dpool.tile([128, 18 * CB], DT)
            E3 = dpool.tile([128, 8 * CB], DT)
            O3 = dpool.tile([128, 8 * CB], DT)
            EO = [(E1, O1), (E2, O2), (E3, O3)]

            def dma_in():
                # two dma_starts per section: halves land on different HWDGE
                # queues and stream in parallel, roughly halving per-section
                # latency and keeping level 1 fed
                for s, su in enumerate(SEC_US):
                    c = 4 * SEC_START[s] * CB
                    wsec = (4 * su + 2) * CB
                    h = (wsec // (2 * CB)) * CB
                    nc.sync.dma_start(out=xsecs[s][:, 0:h],
                                      in_=xa_d[:, c:c + h])
                    nc.sync.dma_start(out=xsecs[s][:, h:wsec],
                                      in_=xa_d[:, c + h:c + wsec])

            def gview(t):
                return t.rearrange("p (g t c) -> p g t c", t=2, c=CB)

            def do_unit(lvl, U, d_dram):
                n_units = N_UNITS[lvl]
                Edst, Odst = EO[lvl]
                if True:
                    if lvl == 0:
                        s = sec_of(U)
                        g0 = 2 * (U - SEC_START[s])
                    else:
                        g0 = 2 * U
                    psE = ppool.tile([128, W], f32, tag="psE")
                    psO = ppool.tile([128, W], f32, tag="psO")
                    if lvl == 0:
                        v = gview(xsecs[s])
                        mv_e = v[:, g0:g0 + 2, 0, :]
                        mv_o = v[:, g0:g0 + 2, 1, :]
                        mv_b = v[:, g0 + 1:g0 + 3, 0, :]
                        nc.tensor.matmul(psE, S_M, mv_e, start=True, stop=False)
                        nc.tensor.matmul(psE, S_B, mv_o, start=False, stop=True)
                        nc.tensor.matmul(psO, S_M, mv_o, start=True, stop=False)
                        nc.tensor.matmul(psO, S_B, mv_b, start=False, stop=True)
                    else:
                        # inputs live split across Eprev (lower halves of the
                        # next-level tiles, partitions 0:64) and Oprev (upper
                        # halves, also at partitions 0:64) -> K=64 matmuls,
                        # all base-0 (mixed-base accumulation is rejected by HW)
                        Ep, Op = EO[lvl - 1]
                        ve = gview(Ep)[0:64, :, :, :]
                        vo = gview(Op)[0:64, :, :, :]
                        e_ev = ve[:, g0:g0 + 2, 0, :]
                        e_od = ve[:, g0:g0 + 2, 1, :]
                        e_sh = ve[:, g0 + 1:g0 + 3, 0, :]
                        o_ev = vo[:, g0:g0 + 2, 0, :]
                        o_od = vo[:, g0:g0 + 2, 1, :]
                        nc.tensor.matmul(psE, S_M[0:64, :], e_ev,
                                         start=True, stop=False)
                        nc.tensor.matmul(psE, S_Mh[0:64, :], o_ev,
                                         start=False, stop=False)
                        nc.tensor.matmul(psE, S_B[0:64, :], e_od,
                                         start=False, stop=True)
                        nc.tensor.matmul(psO, S_M[0:64, :], e_od,
                                         start=True, stop=False)
                        nc.tensor.matmul(psO, S_Mh[0:64, :], o_od,
                                         start=False, stop=False)
                        nc.tensor.matmul(psO, S_B[0:64, :], e_sh,
                                         start=False, stop=True)
                    c0 = W * U
                    # one full-width evacuation per psum tile (psum-read cost
                    # is per free-col: never split a psum copy by partitions)
                    nc.vector.tensor_copy(out=Edst[:, c0:c0 + W], in_=psE[:])
                    nc.scalar.copy(out=Odst[:, c0:c0 + W], in_=psO[:])
                    if lvl < 2 and U == 0:   # circular wrap pad block
                        cp = CB * (2 * n_units)
                        nc.vector.tensor_copy(out=Edst[:, cp:cp + CB],
                                              in_=psE[:, 0:CB])
                        nc.scalar.copy(out=Odst[:, cp:cp + CB],
                                       in_=psO[:, 0:CB])
                    # outputs stream out per U-pair (levels 1-2, in-order) or
                    # per unit (level 3, whose units are emitted out of order)
                    if d_dram is not None:
                        if U % 2 == 1:
                            c1 = W * (U - 1)
                            nc.sync.dma_start(out=d_dram[64:128, c1:c1 + 2 * W],
                                              in_=Edst[64:128, c1:c1 + 2 * W])
                            nc.sync.dma_start(out=d_dram[0:64, c1:c1 + 2 * W],
                                              in_=Odst[64:128, c1:c1 + 2 * W])
                    else:
                        nc.sync.dma_start(out=e3_d[:, c0:c0 + W],
                                          in_=Edst[:, c0:c0 + W])
                        nc.sync.dma_start(out=o3_d[:, c0:c0 + W],
                                          in_=Odst[:, c0:c0 + W])

            wtile = cpool.tile([128, 128], DT)

            def warmup():
                # PE warmup (depends only on a memset): dummy matmuls ramp the
                # clock gate to full rate while input sections stream in.
                nc.vector.memset(wtile, 0.0)
                wps = ppool.tile([128, W], f32, tag="psE", name="wps")
                for _ in range(14):
                    nc.tensor.matmul(wps[:, 0:128], wtile, wtile,
                                     start=True, stop=True)

            def whole():
                dma_in()
                # interleave levels: a unit is emitted as soon as its inputs
                # exist, so levels 2/3 fill level-1's DMA-stall gaps and the
                # kernel tail stays short. L3-0 goes last (its inputs are
                # ready long before) to keep the final dependency chain flat.
                for U in range(N_UNITS[0]):
                    do_unit(0, U, d1_d)
                    if U >= 2 and U % 2 == 0:
                        U2 = (U - 2) // 2
                        do_unit(1, U2, d2_d)
                        if U2 in (4, 6):
                            do_unit(2, U2 // 2 - 1, None)
                do_unit(1, 7, d2_d)
                do_unit(2, 3, None)
                do_unit(2, 0, None)

            warmup()
            if hw_loop:
                # warmup sits before the loop: iterations keep the PE warm on
                # their own, so the loop slope matches steady-state time
                with tc.For_i(0, hw_loop, 1):
                    whole()
            else:
                for _ in range(repeat):
                    whole()

    nc.compile()
    return nc


_BUILD_CACHE = {}


def _get_runner(repeat=1, hw_loop=0):
    """Build (once) and return a jitted SPMD runner: fn(in_maps) -> results."""
    key = (MM_DTYPE, repeat, hw_loop)
    if key in _BUILD_CACHE:
        return _BUILD_CACHE[key]

    import jax
    from jax.sharding import Mesh, PartitionSpec
    from jax.experimental.shard_map import shard_map

    nc = build_bass(repeat, hw_loop)
    bass2jax.install_neuronx_cc_hook()

    partition_name = nc.partition_id_tensor.name if nc.partition_id_tensor else None
    in_names, out_names, out_avals, zero_outs = [], [], [], []
    for alloc in nc.m.functions[0].allocations:
        if not isinstance(alloc, mybir.MemoryLocationSet):
            continue
        name = alloc.memorylocations[0].name
        if alloc.kind == "ExternalInput":
            if name != partition_name:
                in_names.append(name)
        elif alloc.kind == "ExternalOutput":
            out_names.append(name)
            shape = tuple(alloc.tensor_shape)
            dtype = mybir.dt.np(alloc.dtype)
            out_avals.append(jax.core.ShapedArray(shape, dtype))
            zero_outs.append(np.zeros(shape, dtype))
    n_params = len(in_names)
    n_outs = len(out_avals)
    all_in_names = list(in_names) + list(out_names)
    if partition_name is not None:
        all_in_names.append(partition_name)
    donate = tuple(range(n_params, n_params + n_outs))

    def _body(*args):
        operands = list(args)
        if partition_name is not None:
            operands.append(bass2jax.partition_id_tensor())
        outs = bass2jax._bass_exec_p.bind(
            *operands,
            out_avals=tuple(out_avals),
            in_names=tuple(all_in_names),
            out_names=tuple(out_names),
            lowering_input_output_aliases=(),
            sim_require_finite=True,
            sim_require_nnan=True,
            nc=nc,
        )
        return tuple(outs)

    devices = jax.devices()[:N_CORES]
    mesh = Mesh(np.asarray(devices), ("core",))
    in_specs = (PartitionSpec("core"),) * (n_params + n_outs)
    out_specs = (PartitionSpec("core"),) * len(out_names)
    sharded = jax.jit(
        shard_map(_body, mesh=mesh, in_specs=in_specs, out_specs=out_specs,
                  check_rep=False),
        donate_argnums=donate, keep_unused=True,
    )

    def run(in_maps, raw=False):
        per_core = [[np.asarray(m[name]) for name in in_names] for m in in_maps]
        concat_in = [np.concatenate([per_core[c][i] for c in range(N_CORES)], axis=0)
                     for i in range(n_params)]
        concat_zeros = [np.zeros((N_CORES * z.shape[0], *z.shape[1:]), z.dtype)
                        for z in zero_outs]
        out_arrs = sharded(*concat_in, *concat_zeros)
        if raw:
            return out_arrs
        return [
            {name: np.asarray(out_arrs[i]).reshape(N_CORES, *out_avals[i].shape)[c]
             for i, name in enumerate(out_names)}
            for c in range(N_CORES)
        ]

    run.sharded = sharded
    run.in_names = in_names
    run.out_names = out_names
    run.out_avals = out_avals
    run.zero_outs = zero_outs
    run.nc = nc
    _BUILD_CACHE[key] = run
    return run


def _prep_core(x2d, np_dt):
    """x2d (8192, CB) fp32 -> xa (128, 66*CB) tile-major + 2 wrap-pad blocks."""
    t = x2d.reshape(64, 128, CB)
    t = np.concatenate([t, t[0:2]], axis=0)      # 66 blocks
    return np.ascontiguousarray(t.transpose(1, 0, 2).reshape(128, 66 * CB)).astype(np_dt)


def _unscramble_d(arr):
    """(128, n_units*512) fp16 scrambled D -> (n_pos, BC, C) fp32 seq-major."""
    n_units = arr.shape[1] // W
    a = arr.astype(np.float32).reshape(128, n_units, 2, CB)
    lo, hi = a[:64], a[64:]          # lo: odd blocks, hi: even blocks
    # out block k=4U+2h+t: t=0 -> hi, t=1 -> lo ; shape (U, h, t, 64, CB)
    blocks = np.stack([hi, lo], axis=3)            # (64, U, 2, 2, CB)
    blocks = blocks.transpose(1, 2, 3, 0, 4)       # (U, h, t, 64, CB)
    seq = blocks.reshape(n_units * 256, BC, C)
    return seq


def _unscramble_ad(e3, o3):
    """Final level mixed psum stages -> (A3_seq, D3_seq) each (n_pos, BC, C)."""
    n_units = e3.shape[1] // W
    e = e3.astype(np.float32).reshape(128, n_units, 2, CB)
    o = o3.astype(np.float32).reshape(128, n_units, 2, CB)
    A_even, D_even = e[:64], e[64:]
    A_odd, D_odd = o[:64], o[64:]
    A = np.stack([A_even, A_odd], axis=3).transpose(1, 2, 3, 0, 4)
    D = np.stack([D_even, D_odd], axis=3).transpose(1, 2, 3, 0, 4)
    return (A.reshape(n_units * 256, BC, C), D.reshape(n_units * 256, BC, C))


def kernel(x):
    x = np.asarray(x, dtype=np.float32)
    assert x.shape == (B, N0, C)
    np_dt = _np_dt(MM_DTYPE)
    smats = make_stationaries().astype(np_dt)

    in_maps = []
    for i in range(N_CORES):
        xc = x[BC * i:BC * (i + 1)]                  # (BC, 8192, C)
        x2d = xc.transpose(1, 0, 2).reshape(N0, CB)  # (seq, cb)
        in_maps.append({"xa": _prep_core(x2d, np_dt), "smats": smats})

    res = _get_runner()(in_maps)

    A3 = np.empty((B, N0 // 8, C), np.float32)
    D3 = np.empty((B, N0 // 8, C), np.float32)
    D2 = np.empty((B, N0 // 4, C), np.float32)
    D1 = np.empty((B, N0 // 2, C), np.float32)
    for i in range(N_CORES):
        sl = slice(BC * i, BC * (i + 1))
        a3s, d3s = _unscramble_ad(np.asarray(res[i]["e3"]), np.asarray(res[i]["o3"]))
        A3[sl] = a3s.transpose(1, 0, 2)
        D3[sl] = d3s.transpose(1, 0, 2)
        D2[sl] = _unscramble_d(np.asarray(res[i]["d2"])).transpose(1, 0, 2)
        D1[sl] = _unscramble_d(np.asarray(res[i]["d1"])).transpose(1, 0, 2)
    return (A3, D3, D2, D1)
